# revision 19
# baseline (speedup 1.0000x reference)
"""Multi-head attention (batch=2, seq=2048, d_model=2048, 16 heads, causal)
on 8 Trainium2 NeuronCores.

Sharding (Megatron-style tensor parallel + data parallel):
  core c -> batch b = c // 4, feature block j = c % 4 (4 heads = 512 features).
  Each core computes Q/K/V projections for its 512 feature columns
  (w_q/w_k/w_v column-sliced), attention for its 4 heads, and a partial
  output projection (w_o row-sliced).  The 4 partial outputs per batch
  element are summed on the host (the Megatron row-parallel AllReduce).

v4 per-head software pipeline: ACT-engine exp (~80us serial) hides under
PE projection matmuls; a single compact triangular T buffer (ring-1)
keeps SBUF under budget.
  S0: Q-h0 + K-h0, r-outer over shared xt chunks (DMA-paced startup)
  S1: V (all heads)  ‖ pass1-h0 woven, pass2-h0 in the tail
  W1: Q-h1 K-h1      ‖ pass2-h0 finish
  W2: Q-h2 K-h2      ‖ pass1-h1
  W3: Q-h3 K-h3      ‖ pass2-h1 + pass1-h2
  W4: pass2-h2       ‖ pass1-h3
  W5: pass2-h3       ‖ output projection (interleaved, split DMA tail)
All matmuls bf16 with fp32 PSUM accumulation; unnormalized softmax with
a fused ones-column denominator in V (scores are O(5), fp32 exp: no max
shift needed).  wq/wk are host-packed per-head ([4*128, 2048]) so each
head's weight slice is one contiguous DMA.
"""

import math
import threading
from contextlib import ExitStack

import ml_dtypes
import numpy as np

import concourse.bass as bass
import concourse.mybir as mybir
import concourse.tile as tile
from concourse import bacc
from concourse.masks import make_identity

BF16 = mybir.dt.bfloat16
F32 = mybir.dt.float32
NPBF16 = ml_dtypes.bfloat16

SEQ = 2048
DM = 2048
HEADS_PER_CORE = 4
F = 512  # features per core
P = 128
NKC = SEQ // P  # 16 key blocks
NR = DM // P  # 16 contraction chunks
SCALE = 1.0 / math.sqrt(128.0)

# compact T-buffer offsets: block kc covers q in [kc*128, 2048)
T_WIDTHS = [SEQ - kc * P for kc in range(NKC)]
T_OFFS = list(np.cumsum([0] + T_WIDTHS[:-1]))
T_TOTAL = int(np.sum(T_WIDTHS))  # 17408


class Weaver:
    """Paced round-robin emitter of chunk closures."""

    def __init__(self):
        self.chunks = []
        self.pos = 0

    def add(self, chunks):
        self.chunks.extend(chunks)

    def emit(self, n=1):
        e = 0
        while self.pos < len(self.chunks) and e < n:
            self.chunks[self.pos]()
            self.pos += 1
            e += 1

    def pending(self):
        return len(self.chunks) - self.pos

    def drain(self):
        self.emit(self.pending())


def build_nc(iters: int = 1, **_ignored) -> bacc.Bacc:
    nc = bacc.Bacc("TRN2", num_devices=8)

    xt_h = nc.dram_tensor("xt", [DM, SEQ], BF16, kind="ExternalInput")
    # wq/wk packed per head: row h*128+p, col r*128+f  <-  w[r*128+p, h*128+f]
    wq_h = nc.dram_tensor("wq", [HEADS_PER_CORE * P, DM], BF16, kind="ExternalInput")
    wk_h = nc.dram_tensor("wk", [HEADS_PER_CORE * P, DM], BF16, kind="ExternalInput")
    wv_h = nc.dram_tensor("wv", [DM, F], BF16, kind="ExternalInput")
    wo_h = nc.dram_tensor("wo", [F, DM], BF16, kind="ExternalInput")
    tri_h = nc.dram_tensor("tri", [P, P], BF16, kind="ExternalInput")
    out_h = nc.dram_tensor("out", [SEQ, DM], BF16, kind="ExternalOutput")

    xt = xt_h.ap()
    wq_ap = wq_h.ap()
    wk_ap = wk_h.ap()
    wo_r = wo_h.ap().rearrange("(c p) n -> p c n", p=P)  # [128, 4, 2048]
    out_ap = out_h.ap()

    with tile.TileContext(nc) as tc, ExitStack() as octx:
        consts = octx.enter_context(tc.tile_pool(name="consts", bufs=1))
        ident = consts.tile([P, P], BF16)
        make_identity(nc, ident)
        tri_sb = consts.tile([P, P], BF16)
        nc.sync.dma_start(out=tri_sb, in_=tri_h.ap())

        for _ in range(iters):
            with ExitStack() as ictx:
                persist = ictx.enter_context(tc.tile_pool(name="persist", bufs=1))
                qt_sb = persist.tile([P, HEADS_PER_CORE, SEQ], BF16)
                kt_sb = persist.tile([P, HEADS_PER_CORE, SEQ], BF16)
                v_sb = persist.tile([P, NKC, HEADS_PER_CORE, P + 1], BF16)
                ot_sb = persist.tile([P, HEADS_PER_CORE, SEQ], BF16)
                nc.vector.memset(v_sb[:, :, :, P : P + 1], 1.0)
                pbo = ictx.enter_context(tc.tile_pool(name="pbo", bufs=3))
                tp = ictx.enter_context(tc.tile_pool(name="tp", bufs=1))
                wqk = ictx.enter_context(tc.tile_pool(name="wqk", bufs=1))
                pa_ctx = ExitStack()
                pa = pa_ctx.enter_context(
                    tc.tile_pool(name="pa", bufs=1, side="right")
                )
                xt_sb = pa.tile([P, NR, SEQ], BF16)

                # ---------------- emitters ----------------
                def pass1_chunks(h, t_t, fps_pool, width=1024):
                    """T = causal_mask(exp(S^T/sqrt(d))), compact kc-major
                    chunks with kt-weight reuse."""
                    chunks = []
                    for kc in range(NKC):
                        w = T_WIDTHS[kc]
                        for c in range((w + width - 1) // width):
                            def emit(kc=kc, c=c, w=w):
                                off = T_OFFS[kc]
                                q0 = kc * P
                                lhsT = kt_sb[:, h, kc * P : (kc + 1) * P]
                                wc = min(width, w - c * width)
                                ps = fps_pool.tile(
                                    [P, width], F32, tag="fps", name="fps"
                                )
                                for n in range((wc + 511) // 512):
                                    nw = min(512, wc - n * 512)
                                    o0 = c * width + n * 512
                                    nc.tensor.matmul(
                                        ps[:, n * 512 : n * 512 + nw],
                                        lhsT,
                                        qt_sb[:, h, q0 + o0 : q0 + o0 + nw],
                                        start=True,
                                        stop=True,
                                    )
                                nc.scalar.activation(
                                    t_t[:, off + c * width : off + c * width + wc],
                                    ps[:, 0:wc],
                                    mybir.ActivationFunctionType.Exp,
                                    scale=SCALE,
                                )
                                if c == 0:
                                    nc.vector.tensor_mul(
                                        t_t[:, off : off + P],
                                        t_t[:, off : off + P],
                                        tri_sb,
                                    )
                            chunks.append(emit)
                    return chunks

                def pass2_groups(h, t_t, pso_pool, pst_pool):
                    """AV + normalize + transpose per query block.
                    pso_pool may be a pool or a qb->pool function."""
                    state = {}
                    groups = []
                    for qb in range(NKC):
                        def emit(qb=qb):
                            pp = pso_pool(qb) if callable(pso_pool) else pso_pool
                            po = pp.tile([P, P + 1], F32, tag="pso", name="po")
                            for kc in range(qb + 1):
                                col = T_OFFS[kc] + (qb - kc) * P
                                nc.tensor.matmul(
                                    po,
                                    t_t[:, col : col + P],
                                    v_sb[:, kc, h, :],
                                    start=(kc == 0),
                                    stop=(kc == qb),
                                )
                            recip = pbo.tile([P, 1], F32, tag="recip", name="recip")
                            nc.vector.reciprocal(recip, po[:, P : P + 1])
                            o_sb = pbo.tile([P, P], BF16, tag="o", name="o_sb")
                            nc.vector.tensor_scalar_mul(o_sb, po[:, 0:P], recip)
                            i4 = qb % 4
                            if i4 == 0:
                                state["pt"] = pst_pool.tile(
                                    [P, 512], BF16, tag="pst", name="pt"
                                )
                            nc.tensor.transpose(
                                state["pt"][:, i4 * P : (i4 + 1) * P], o_sb, ident
                            )
                            if i4 == 3:
                                g = qb // 4
                                nc.vector.tensor_copy(
                                    ot_sb[:, h, g * 512 : (g + 1) * 512],
                                    state["pt"],
                                )
                        groups.append(emit)
                    return groups

                def qk_head(h, w_t, dst, proj_pool, weaver, per_slot=1):
                    """Steady-state projection for one head: 1024-col halves,
                    r-inner, weaving attention chunks between r groups."""
                    for half in range(2):
                        pk = proj_pool.tile([P, 1024], F32, tag="proj", name="pk")
                        s0 = half * 1024
                        for r in range(NR):
                            lhsT = w_t[:, r * P : (r + 1) * P]
                            for sn in range(2):
                                nc.tensor.matmul(
                                    pk[:, sn * 512 : (sn + 1) * 512],
                                    lhsT,
                                    xt_sb[:, r, s0 + sn * 512 : s0 + (sn + 1) * 512],
                                    start=(r == 0),
                                    stop=(r == NR - 1),
                                )
                            if r % 3 == 2:
                                weaver.emit(per_slot)
                        nc.vector.tensor_copy(dst[:, h, s0 : s0 + 1024], pk)
                        weaver.emit(per_slot)

                # ---------------- DMA schedule ----------------
                wq_t = [None] * HEADS_PER_CORE
                wk_t = [None] * HEADS_PER_CORE

                def fetch_qk(h):
                    wq_t[h] = wqk.tile([P, DM], BF16, tag="wq", name="wq_t")
                    nc.sync.dma_start(
                        out=wq_t[h], in_=wq_ap[h * P : (h + 1) * P, :]
                    )
                    wk_t[h] = wqk.tile([P, DM], BF16, tag="wk", name="wk_t")
                    nc.sync.dma_start(
                        out=wk_t[h], in_=wk_ap[h * P : (h + 1) * P, :]
                    )

                # wq -> xt0 -> wk, then per-chunk xt DMAs
                wq_t[0] = wqk.tile([P, DM], BF16, tag="wq", name="wq_t")
                nc.sync.dma_start(out=wq_t[0], in_=wq_ap[0:P, :])
                nc.sync.dma_start(out=xt_sb[:, 0, :], in_=xt[0:P, :])
                wk_t[0] = wqk.tile([P, DM], BF16, tag="wk", name="wk_t")
                nc.sync.dma_start(out=wk_t[0], in_=wk_ap[0:P, :])
                for r in range(1, NR):
                    nc.sync.dma_start(
                        out=xt_sb[:, r, :], in_=xt[r * P : (r + 1) * P, :]
                    )

                # ======== S0: Q-h0 full + K-h0 half0, r-outer (DMA-paced) ====
                # Q psum on the left PSUM stack, K-half on the right: V's psum
                # (right, below) then never overlaps a zone gated on S0 copies,
                # so V starts with no seam.  K-half1 is woven into V.
                with ExitStack() as s0ctx:
                    ps0q = s0ctx.enter_context(
                        tc.tile_pool(name="ps0q", bufs=1, space="PSUM")
                    )
                    ps0k = s0ctx.enter_context(
                        tc.tile_pool(name="ps0k", bufs=1, space="PSUM", side="right")
                    )
                    pq = ps0q.tile([P, SEQ], F32, tag="ps0q", name="pq")
                    pk0 = ps0k.tile([P, 1024], F32, tag="ps0k", name="pk0")
                    for r in range(NR):
                        lhsT = wq_t[0][:, r * P : (r + 1) * P]
                        for sn in range(4):
                            nc.tensor.matmul(
                                pq[:, sn * 512 : (sn + 1) * 512],
                                lhsT,
                                xt_sb[:, r, sn * 512 : (sn + 1) * 512],
                                start=(r == 0),
                                stop=(r == NR - 1),
                            )
                        lhsT = wk_t[0][:, r * P : (r + 1) * P]
                        for sn in range(2):
                            nc.tensor.matmul(
                                pk0[:, sn * 512 : (sn + 1) * 512],
                                lhsT,
                                xt_sb[:, r, sn * 512 : (sn + 1) * 512],
                                start=(r == 0),
                                stop=(r == NR - 1),
                            )
                    # kt first (pass1-h0 needs it)
                    nc.vector.tensor_copy(kt_sb[:, 0, 0:1024], pk0)
                    nc.vector.tensor_copy(qt_sb[:, 0, 0:1024], pq[:, 0:1024])
                    nc.scalar.copy(qt_sb[:, 0, 1024:2048], pq[:, 1024:2048])

                # attention-era PSUM pools (LIFO: pso, pst, fps | psv | proj | pco)
                att_ctx = ExitStack()
                pso = att_ctx.enter_context(
                    tc.tile_pool(name="pso", bufs=1, space="PSUM")
                )
                pst = att_ctx.enter_context(
                    tc.tile_pool(name="pst", bufs=1, space="PSUM")
                )
                fps_ctx = ExitStack()
                fps = fps_ctx.enter_context(
                    tc.tile_pool(name="fps", bufs=1, space="PSUM")
                )

                # two T buffers alternate across heads (h0,h2 -> tA; h1,h3 ->
                # tB): pass1-h(i+1) then never overwrites what pass2-h(i) is
                # still reading
                t_a = tp.tile([P, T_TOTAL], BF16, tag="T", name="t_a", bufs=1)
                weaver = Weaver()

                # ======== S1: V (all heads) ‖ K-h0 half1 ‖ pass1-h0 ========
                with ExitStack() as vctx:
                    psk1 = vctx.enter_context(
                        tc.tile_pool(name="psk1", bufs=1, space="PSUM", side="right")
                    )
                    psv = vctx.enter_context(
                        tc.tile_pool(name="psv", bufs=2, space="PSUM", side="right")
                    )
                    wvp = vctx.enter_context(tc.tile_pool(name="wvp", bufs=1))

                    kh1_state = {}

                    def kh1_chunk(g):
                        def emit():
                            if g == 0:
                                kh1_state["pk"] = psk1.tile(
                                    [P, 1024], F32, tag="psk1", name="pk1"
                                )
                            pk1 = kh1_state["pk"]
                            for r in range(4 * g, 4 * g + 4):
                                lhsT = wk_t[0][:, r * P : (r + 1) * P]
                                for sn in range(2):
                                    nc.tensor.matmul(
                                        pk1[:, sn * 512 : (sn + 1) * 512],
                                        lhsT,
                                        xt_sb[
                                            :, r, 1024 + sn * 512 : 1024 + (sn + 1) * 512
                                        ],
                                        start=(r == 0),
                                        stop=(r == NR - 1),
                                    )
                            if g == 3:
                                nc.vector.tensor_copy(
                                    kt_sb[:, 0, 1024:2048], pk1
                                )
                        return emit

                    p1h0 = pass1_chunks(0, t_a, fps)
                    inter0 = []
                    for g in range(4):
                        inter0.append(kh1_chunk(g))
                        inter0 += p1h0[4 * g : 4 * g + 4]
                    inter0 += p1h0[16:]
                    weaver.add(inter0)
                    wv_r = wv_h.ap().rearrange("(r p) f -> p r f", p=P)
                    wv_t = wvp.tile([P, NR, F], BF16)
                    for rr in range(4):
                        nc.sync.dma_start(
                            out=wv_t[:, rr * 4 : (rr + 1) * 4, :],
                            in_=wv_r[:, rr * 4 : (rr + 1) * 4, :],
                        )
                    # prefetch weights for heads 1-2 (ring-gated on h0/h1 release;
                    # queued after wv/wo so they can't head-of-line-block them)
                    fetch_qk(1)
                    fetch_qk(2)

                    p2h0 = pass2_groups(0, t_a, pso, pst)
                    for sm in range(NKC):
                        pv = psv.tile([P, F], F32, tag="psv", name="pv")
                        for r in range(NR):
                            nc.tensor.matmul(
                                pv,
                                xt_sb[:, r, sm * P : (sm + 1) * P],
                                wv_t[:, r, :],
                                start=(r == 0),
                                stop=(r == NR - 1),
                            )
                            if r == 7:
                                weaver.emit(1)
                        nc.vector.tensor_copy(
                            v_sb[:, sm, :, 0:P],
                            pv.rearrange("p (h d) -> p h d", h=HEADS_PER_CORE),
                        )
                        weaver.emit(1)
                        if sm == 10:
                            weaver.add(p2h0)

                # ======== W1-W3: Q/K heads 1-3 ‖ attention pipeline ========
                tpB = ictx.enter_context(tc.tile_pool(name="tpB", bufs=1))
                with ExitStack() as wctx:
                    proj = wctx.enter_context(
                        tc.tile_pool(name="proj", bufs=2, space="PSUM")
                    )
                    # W1: Q1 K1 ‖ finish pass2-h0
                    qk_head(1, wq_t[1], qt_sb, proj, weaver)
                    qk_head(1, wk_t[1], kt_sb, proj, weaver)
                    weaver.drain()  # anything left of h0
                    # W2: Q2 K2 ‖ pass1-h1
                    t_b = tpB.tile([P, T_TOTAL], BF16, name="t_b")
                    weaver.add(pass1_chunks(1, t_b, fps))
                    fetch_qk(3)
                    qk_head(2, wq_t[2], qt_sb, proj, weaver)
                    qk_head(2, wk_t[2], kt_sb, proj, weaver)
                    # W3: Q3 K3 ‖ pass2-h1 + pass1-h2 (interleaved 1:2)
                    p2h1 = pass2_groups(1, t_b, pso, pst)
                    p1h2 = pass1_chunks(2, t_a, fps)
                    inter = []
                    i1 = i2 = 0
                    while i1 < len(p2h1) or i2 < len(p1h2):
                        if i2 < len(p1h2):
                            inter.append(p1h2[i2]); i2 += 1
                        if i2 < len(p1h2):
                            inter.append(p1h2[i2]); i2 += 1
                        if i1 < len(p2h1):
                            inter.append(p2h1[i1]); i1 += 1
                    weaver.add(inter)
                    qk_head(3, wq_t[3], qt_sb, proj, weaver, per_slot=2)
                    qk_head(3, wk_t[3], kt_sb, proj, weaver, per_slot=2)
                    weaver.drain()
                pa_ctx.close()  # xt no longer needed
                # wo staged here: its SBUF slot only fits after xt is freed
                cw = ictx.enter_context(tc.tile_pool(name="cw", bufs=1))
                wo_sb = cw.tile([P, HEADS_PER_CORE, DM], BF16)
                nc.sync.dma_start(out=wo_sb[:, 0:2, :], in_=wo_r[:, 0:2, :])
                nc.sync.dma_start(out=wo_sb[:, 2:4, :], in_=wo_r[:, 2:4, :])

                # ======== W4: pass2-h2 ‖ pass1-h3 ========
                with ExitStack() as w4ctx:
                    fps2 = w4ctx.enter_context(
                        tc.tile_pool(name="fps2", bufs=1, space="PSUM")
                    )
                    psoB = w4ctx.enter_context(
                        tc.tile_pool(name="psoB", bufs=2, space="PSUM")
                    )

                    class AltPool:
                        # alternate chunks between the still-open fps pool and
                        # fps2: double-buffering across pools, and chunk 0 never
                        # waits on the proj-pool bank release at the W3/W4 seam
                        def __init__(self):
                            self.i = 0

                        def tile(self, *a, **kw):
                            self.i += 1
                            return (fps if self.i % 2 else fps2).tile(*a, **kw)

                    # first half of the groups use the already-open pso ring so
                    # nothing waits on the proj-pool release at the W3/W4 seam
                    p2h2 = pass2_groups(
                        2, t_a, lambda qb: pso if qb < 4 else psoB, pst
                    )
                    p1h3 = pass1_chunks(3, t_b, AltPool())
                    i1 = i2 = 0
                    while i1 < len(p2h2) or i2 < len(p1h3):
                        if i1 < len(p2h2):
                            p2h2[i1](); i1 += 1
                        while i2 < len(p1h3) and i2 * len(p2h2) <= i1 * len(p1h3):
                            p1h3[i2](); i2 += 1

                fps_ctx.close()

                # ======== W5: pass2-h3 ‖ output projection ========
                with ExitStack() as cctx:
                    psoC = cctx.enter_context(
                        tc.tile_pool(name="psoC", bufs=2, space="PSUM")
                    )
                    pco = cctx.enter_context(
                        tc.tile_pool(name="pco", bufs=2, space="PSUM")
                    )
                    stg = cctx.enter_context(tc.tile_pool(name="stg", bufs=3))

                    c_stage = {}

                    def c_half(sm, half, split=1):
                        po = pco.tile([P, 1024], F32, tag="pco", name="pco_t")
                        for ff in range(HEADS_PER_CORE):
                            lhsT = ot_sb[:, ff, sm * P : (sm + 1) * P]
                            for nd in range(2):
                                n0 = half * 1024 + nd * 512
                                nc.tensor.matmul(
                                    po[:, nd * 512 : (nd + 1) * 512],
                                    lhsT,
                                    wo_sb[:, ff, n0 : n0 + 512],
                                    start=(ff == 0),
                                    stop=(ff == HEADS_PER_CORE - 1),
                                )
                        # both halves stage into one row tile: 1 out-DMA per sm
                        # (halves the DMA-setup count; the queue drains faster)
                        if sm not in c_stage:
                            c_stage[sm] = stg.tile(
                                [P, DM], BF16, tag="stage", name="stage"
                            )
                        seg = c_stage[sm][:, half * 1024 : (half + 1) * 1024]
                        if half == 0:
                            nc.vector.tensor_copy(seg, po)
                        else:
                            nc.scalar.copy(seg, po)
                        if split > 1:
                            # final row: each half DMAs itself right after its
                            # own copy, shortening the drain tail
                            nc.scalar.dma_start(
                                out=out_ap[
                                    sm * P : (sm + 1) * P,
                                    half * 1024 : (half + 1) * 1024,
                                ],
                                in_=seg,
                            )
                        elif half == 1:
                            nc.scalar.dma_start(
                                out=out_ap[sm * P : (sm + 1) * P, :],
                                in_=c_stage[sm],
                            )

                    # one eligible C-half after every pass2-h3 group keeps the
                    # PE fed between groups (pso ring); 2 halves unlock per quad
                    p2h3 = pass2_groups(3, t_b, psoC, pst)
                    ready = []
                    for qb in range(NKC):
                        p2h3[qb]()
                        if qb % 4 == 3:
                            sm = qb // 4
                            ready += [(sm, 0), (sm, 1)]
                        if ready:
                            c_half(*ready.pop(0))
                    for sm, half in ready:
                        c_half(sm, half)
                    for u in range(8, 2 * NKC):
                        c_half(u // 2, u % 2, split=2 if u >= 2 * NKC - 2 else 1)
                att_ctx.close()

    nc.compile()
    return nc


def prep_in_maps(x, mask, w_q, w_k, w_v, w_o):
    """Host-side sharding: per-core input dicts (8 cores)."""
    x = np.asarray(x, dtype=np.float32)
    mask = np.asarray(mask, dtype=np.float32)
    w_q = np.asarray(w_q, dtype=np.float32)
    w_k = np.asarray(w_k, dtype=np.float32)
    w_v = np.asarray(w_v, dtype=np.float32)
    w_o = np.asarray(w_o, dtype=np.float32)

    # tri[k, q] = 1 where allowed (k <= q), from the mask's diagonal block
    tri = np.ascontiguousarray(
        (mask[:P, :P].T == 0.0).astype(NPBF16)
    )
    xts = [np.ascontiguousarray(x[b].T).astype(NPBF16) for b in range(2)]

    def pack_heads(w):  # [DM, F] -> [4*128, DM]: row h*128+p <- w[r*128+p, h*128+f]
        a = w.reshape(NR, P, HEADS_PER_CORE, P)  # [r, p, h, f]
        return np.ascontiguousarray(
            a.transpose(2, 1, 0, 3).reshape(HEADS_PER_CORE * P, DM)
        )

    in_maps = []
    for c in range(8):
        b, j = divmod(c, 4)
        sl = slice(j * F, (j + 1) * F)
        in_maps.append(
            {
                "xt": xts[b],
                "wq": pack_heads(w_q[:, sl]).astype(NPBF16),
                "wk": pack_heads(w_k[:, sl]).astype(NPBF16),
                "wv": np.ascontiguousarray(w_v[:, sl]).astype(NPBF16),
                "wo": np.ascontiguousarray(w_o[sl, :]).astype(NPBF16),
                "tri": tri,
            }
        )
    return in_maps


def gather(results):
    """Sum the 4 partial outputs per batch element."""
    out = np.zeros((2, SEQ, DM), np.float32)
    for c in range(8):
        out[c // 4] += results[c]["out"]
    return out


_cache = threading.local()


def kernel(x, mask, w_q, w_k, w_v, w_o):
    from concourse.bass_utils import run_bass_kernel_spmd

    nc = getattr(_cache, "nc", None)
    if nc is None:
        nc = build_nc(1)
        _cache.nc = nc
    in_maps = prep_in_maps(x, mask, w_q, w_k, w_v, w_o)
    res = run_bass_kernel_spmd(nc, in_maps, core_ids=list(range(8)))
    return gather(res.results)



# revision 26
# speedup vs baseline: 1.2054x; 1.2054x over previous
"""Multi-head attention (batch=2, seq=2048, d_model=2048, 16 heads, causal)
on 8 Trainium2 NeuronCores.

Sharding (Megatron-style tensor parallel + data parallel):
  core c -> batch b = c // 4, feature block j = c % 4 (4 heads = 512 features).
  Each core computes Q/K/V projections for its 512 feature columns
  (w_q/w_k/w_v column-sliced), attention for its 4 heads, and a partial
  output projection (w_o row-sliced).  The 4 partial outputs per batch
  element are summed on the host (the Megatron row-parallel AllReduce).

v4 per-head software pipeline: ACT-engine exp (~80us serial) hides under
PE projection matmuls; a single compact triangular T buffer (ring-1)
keeps SBUF under budget.
  S0: Q-h0 + K-h0, r-outer over shared xt chunks (DMA-paced startup)
  S1: V (all heads)  ‖ pass1-h0 woven, pass2-h0 in the tail
  W1: Q-h1 K-h1      ‖ pass2-h0 finish
  W2: Q-h2 K-h2      ‖ pass1-h1
  W3: Q-h3 K-h3      ‖ pass2-h1 + pass1-h2
  W4: pass2-h2       ‖ pass1-h3
  W5: pass2-h3       ‖ output projection (interleaved, split DMA tail)
All matmuls bf16 with fp32 PSUM accumulation; unnormalized softmax with
a fused ones-column denominator in V (scores are O(5), fp32 exp: no max
shift needed).  wq/wk are host-packed per-head ([4*128, 2048]) so each
head's weight slice is one contiguous DMA.
"""

import math
import threading
from contextlib import ExitStack

import ml_dtypes
import numpy as np

import concourse.bass as bass
import concourse.mybir as mybir
import concourse.tile as tile
from concourse import bacc
from concourse.masks import make_identity

BF16 = mybir.dt.bfloat16
F32 = mybir.dt.float32
NPBF16 = ml_dtypes.bfloat16

SEQ = 2048
DM = 2048
HEADS_PER_CORE = 4
F = 512  # features per core
P = 128
NKC = SEQ // P  # 16 key blocks
NR = DM // P  # 16 contraction chunks
SCALE = 1.0 / math.sqrt(128.0)

# compact T-buffer offsets: block kc covers q in [kc*128, 2048)
T_WIDTHS = [SEQ - kc * P for kc in range(NKC)]
T_OFFS = list(np.cumsum([0] + T_WIDTHS[:-1]))
T_TOTAL = int(np.sum(T_WIDTHS))  # 17408


class Weaver:
    """Paced round-robin emitter of chunk closures."""

    def __init__(self):
        self.chunks = []
        self.pos = 0

    def add(self, chunks):
        self.chunks.extend(chunks)

    def emit(self, n=1):
        e = 0
        while self.pos < len(self.chunks) and e < n:
            self.chunks[self.pos]()
            self.pos += 1
            e += 1

    def pending(self):
        return len(self.chunks) - self.pos

    def drain(self):
        self.emit(self.pending())


def build_nc(iters: int = 1, **_ignored) -> bacc.Bacc:
    nc = bacc.Bacc("TRN2", num_devices=8)

    xt_h = nc.dram_tensor("xt", [DM, SEQ], BF16, kind="ExternalInput")
    # wq/wk packed per head: row h*128+p, col r*128+f  <-  w[r*128+p, h*128+f]
    wq_h = nc.dram_tensor("wq", [HEADS_PER_CORE * P, DM], BF16, kind="ExternalInput")
    wk_h = nc.dram_tensor("wk", [HEADS_PER_CORE * P, DM], BF16, kind="ExternalInput")
    wv_h = nc.dram_tensor("wv", [DM, F], BF16, kind="ExternalInput")
    wo_h = nc.dram_tensor("wo", [F, DM], BF16, kind="ExternalInput")
    tri_h = nc.dram_tensor("tri", [P, P], BF16, kind="ExternalInput")
    out_h = nc.dram_tensor("out", [SEQ, DM], BF16, kind="ExternalOutput")

    xt = xt_h.ap()
    wq_ap = wq_h.ap()
    wk_ap = wk_h.ap()
    wo_r = wo_h.ap().rearrange("(c p) n -> p c n", p=P)  # [128, 4, 2048]
    out_ap = out_h.ap()

    with tile.TileContext(nc) as tc, ExitStack() as octx:
        consts = octx.enter_context(tc.tile_pool(name="consts", bufs=1))
        ident = consts.tile([P, P], BF16)
        make_identity(nc, ident)
        tri_sb = consts.tile([P, P], BF16)
        nc.sync.dma_start(out=tri_sb, in_=tri_h.ap())

        for _ in range(iters):
            with ExitStack() as ictx:
                persist = ictx.enter_context(tc.tile_pool(name="persist", bufs=1))
                qt_sb = persist.tile([P, HEADS_PER_CORE, SEQ], BF16)
                kt_sb = persist.tile([P, HEADS_PER_CORE, SEQ], BF16)
                v_sb = persist.tile([P, NKC, HEADS_PER_CORE, P + 1], BF16)
                ot_sb = persist.tile([P, HEADS_PER_CORE, SEQ], BF16)
                nc.vector.memset(v_sb[:, :, :, P : P + 1], 1.0)
                pbo = ictx.enter_context(tc.tile_pool(name="pbo", bufs=3))
                tp = ictx.enter_context(tc.tile_pool(name="tp", bufs=1))
                wqk = ictx.enter_context(tc.tile_pool(name="wqk", bufs=1))
                pa_ctx = ExitStack()
                pa = pa_ctx.enter_context(
                    tc.tile_pool(name="pa", bufs=1, side="right")
                )
                xt_sb = pa.tile([P, NR, SEQ], BF16)

                # ---------------- emitters ----------------
                def pass1_chunks(h, t_t, fps_pool, width=1024):
                    """T = causal_mask(exp(S^T/sqrt(d))), compact kc-major
                    chunks with kt-weight reuse."""
                    chunks = []
                    for kc in range(NKC):
                        w = T_WIDTHS[kc]
                        for c in range((w + width - 1) // width):
                            def emit(kc=kc, c=c, w=w):
                                off = T_OFFS[kc]
                                q0 = kc * P
                                lhsT = kt_sb[:, h, kc * P : (kc + 1) * P]
                                wc = min(width, w - c * width)
                                ps = fps_pool.tile(
                                    [P, width], F32, tag="fps", name="fps"
                                )
                                for n in range((wc + 511) // 512):
                                    nw = min(512, wc - n * 512)
                                    o0 = c * width + n * 512
                                    nc.tensor.matmul(
                                        ps[:, n * 512 : n * 512 + nw],
                                        lhsT,
                                        qt_sb[:, h, q0 + o0 : q0 + o0 + nw],
                                        start=True,
                                        stop=True,
                                    )
                                nc.scalar.activation(
                                    t_t[:, off + c * width : off + c * width + wc],
                                    ps[:, 0:wc],
                                    mybir.ActivationFunctionType.Exp,
                                    scale=SCALE,
                                )
                                if c == 0:
                                    nc.vector.tensor_mul(
                                        t_t[:, off : off + P],
                                        t_t[:, off : off + P],
                                        tri_sb,
                                    )
                            chunks.append(emit)
                    return chunks

                def pass2_groups(h, t_t, pso_pool, pst_pool):
                    """AV + normalize + transpose per query block.
                    pso_pool may be a pool or a qb->pool function."""
                    state = {}
                    groups = []
                    for qb in range(NKC):
                        def emit(qb=qb):
                            pp = pso_pool(qb) if callable(pso_pool) else pso_pool
                            po = pp.tile([P, P + 1], F32, tag="pso", name="po")
                            for kc in range(qb + 1):
                                col = T_OFFS[kc] + (qb - kc) * P
                                nc.tensor.matmul(
                                    po,
                                    t_t[:, col : col + P],
                                    v_sb[:, kc, h, :],
                                    start=(kc == 0),
                                    stop=(kc == qb),
                                )
                            recip = pbo.tile([P, 1], F32, tag="recip", name="recip")
                            nc.vector.reciprocal(recip, po[:, P : P + 1])
                            o_sb = pbo.tile([P, P], BF16, tag="o", name="o_sb")
                            nc.vector.tensor_scalar_mul(o_sb, po[:, 0:P], recip)
                            i4 = qb % 4
                            if i4 == 0:
                                state["pt"] = pst_pool.tile(
                                    [P, 512], BF16, tag="pst", name="pt"
                                )
                            nc.tensor.transpose(
                                state["pt"][:, i4 * P : (i4 + 1) * P], o_sb, ident
                            )
                            if i4 == 3:
                                g = qb // 4
                                nc.vector.tensor_copy(
                                    ot_sb[:, h, g * 512 : (g + 1) * 512],
                                    state["pt"],
                                )
                        groups.append(emit)
                    return groups

                def qk_head(h, w_t, dst, proj_pool, weaver, per_slot=1):
                    """Steady-state projection for one head: 512-col quarters
                    (1-bank psum tiles), r-inner, weaving attention chunks
                    between r groups."""
                    for quarter in range(4):
                        pk = proj_pool.tile([P, 512], F32, tag="proj", name="pk")
                        s0 = quarter * 512
                        for r in range(NR):
                            nc.tensor.matmul(
                                pk,
                                w_t[:, r * P : (r + 1) * P],
                                xt_sb[:, r, s0 : s0 + 512],
                                start=(r == 0),
                                stop=(r == NR - 1),
                            )
                            if r % 5 == 4:
                                weaver.emit(per_slot)
                        nc.vector.tensor_copy(dst[:, h, s0 : s0 + 512], pk)
                        weaver.emit(per_slot)

                # ---------------- DMA schedule ----------------
                wq_t = [None] * HEADS_PER_CORE
                wk_t = [None] * HEADS_PER_CORE

                def fetch_qk(h):
                    wq_t[h] = wqk.tile([P, DM], BF16, tag="wq", name="wq_t")
                    nc.sync.dma_start(
                        out=wq_t[h], in_=wq_ap[h * P : (h + 1) * P, :]
                    )
                    wk_t[h] = wqk.tile([P, DM], BF16, tag="wk", name="wk_t")
                    nc.sync.dma_start(
                        out=wk_t[h], in_=wk_ap[h * P : (h + 1) * P, :]
                    )

                # wq -> xt0 -> wk, then per-chunk xt DMAs
                wq_t[0] = wqk.tile([P, DM], BF16, tag="wq", name="wq_t")
                nc.sync.dma_start(out=wq_t[0], in_=wq_ap[0:P, :])
                nc.sync.dma_start(out=xt_sb[:, 0, :], in_=xt[0:P, :])
                wk_t[0] = wqk.tile([P, DM], BF16, tag="wk", name="wk_t")
                nc.sync.dma_start(out=wk_t[0], in_=wk_ap[0:P, :])
                for r in range(1, NR):
                    nc.sync.dma_start(
                        out=xt_sb[:, r, :], in_=xt[r * P : (r + 1) * P, :]
                    )

                # ======== S0: Q-h0 full + K-h0 half0, r-outer (DMA-paced) ====
                # Q psum on the left PSUM stack, K-half on the right: V's psum
                # (right, below) then never overlaps a zone gated on S0 copies,
                # so V starts with no seam.  K-half1 is woven into V.
                with ExitStack() as s0ctx:
                    ps0q = s0ctx.enter_context(
                        tc.tile_pool(name="ps0q", bufs=1, space="PSUM")
                    )
                    ps0k = s0ctx.enter_context(
                        tc.tile_pool(name="ps0k", bufs=1, space="PSUM", side="right")
                    )
                    pq = ps0q.tile([P, SEQ], F32, tag="ps0q", name="pq")
                    pk0 = ps0k.tile([P, 1024], F32, tag="ps0k", name="pk0")
                    for r in range(NR):
                        lhsT = wq_t[0][:, r * P : (r + 1) * P]
                        for sn in range(4):
                            nc.tensor.matmul(
                                pq[:, sn * 512 : (sn + 1) * 512],
                                lhsT,
                                xt_sb[:, r, sn * 512 : (sn + 1) * 512],
                                start=(r == 0),
                                stop=(r == NR - 1),
                            )
                        lhsT = wk_t[0][:, r * P : (r + 1) * P]
                        for sn in range(2):
                            nc.tensor.matmul(
                                pk0[:, sn * 512 : (sn + 1) * 512],
                                lhsT,
                                xt_sb[:, r, sn * 512 : (sn + 1) * 512],
                                start=(r == 0),
                                stop=(r == NR - 1),
                            )
                    # kt first (pass1-h0 needs it)
                    nc.vector.tensor_copy(kt_sb[:, 0, 0:1024], pk0)
                    nc.vector.tensor_copy(qt_sb[:, 0, 0:1024], pq[:, 0:1024])
                    nc.scalar.copy(qt_sb[:, 0, 1024:2048], pq[:, 1024:2048])

                # attention-era PSUM pools; pso/pst open after S1 (both
                # ring-2: po and pst chains each double-buffered)
                att_ctx = ExitStack()
                fps_ctx = ExitStack()
                fps = fps_ctx.enter_context(
                    tc.tile_pool(name="fps", bufs=1, space="PSUM", side="right")
                )

                # two T buffers alternate across heads (h0,h2 -> tA; h1,h3 ->
                # tB): pass1-h(i+1) then never overwrites what pass2-h(i) is
                # still reading
                t_a = tp.tile([P, T_TOTAL], BF16, tag="T", name="t_a", bufs=1)
                weaver = Weaver()

                # ======== S1: V (all heads) ‖ K-h0 half1 ‖ pass1-h0 ========
                with ExitStack() as vctx:
                    psk1 = vctx.enter_context(
                        tc.tile_pool(name="psk1", bufs=1, space="PSUM", side="right")
                    )
                    psv = vctx.enter_context(
                        tc.tile_pool(name="psv", bufs=2, space="PSUM", side="right")
                    )
                    wvp = vctx.enter_context(tc.tile_pool(name="wvp", bufs=1))

                    kh1_state = {}

                    def kh1_chunk(g):
                        def emit():
                            if g == 0:
                                kh1_state["pk"] = psk1.tile(
                                    [P, 1024], F32, tag="psk1", name="pk1"
                                )
                            pk1 = kh1_state["pk"]
                            for r in range(4 * g, 4 * g + 4):
                                lhsT = wk_t[0][:, r * P : (r + 1) * P]
                                for sn in range(2):
                                    nc.tensor.matmul(
                                        pk1[:, sn * 512 : (sn + 1) * 512],
                                        lhsT,
                                        xt_sb[
                                            :, r, 1024 + sn * 512 : 1024 + (sn + 1) * 512
                                        ],
                                        start=(r == 0),
                                        stop=(r == NR - 1),
                                    )
                            if g == 3:
                                nc.vector.tensor_copy(
                                    kt_sb[:, 0, 1024:2048], pk1
                                )
                        return emit

                    p1h0 = pass1_chunks(0, t_a, fps)
                    inter0 = []
                    for g in range(4):
                        inter0.append(kh1_chunk(g))
                        inter0 += p1h0[4 * g : 4 * g + 4]
                    inter0 += p1h0[16:]
                    weaver.add(inter0)
                    wv_r = wv_h.ap().rearrange("(r p) f -> p r f", p=P)
                    wv_t = wvp.tile([P, NR, F], BF16)
                    for rr in range(4):
                        nc.sync.dma_start(
                            out=wv_t[:, rr * 4 : (rr + 1) * 4, :],
                            in_=wv_r[:, rr * 4 : (rr + 1) * 4, :],
                        )
                    # prefetch weights for heads 1-2 (ring-gated on h0/h1 release;
                    # queued after wv/wo so they can't head-of-line-block them)
                    fetch_qk(1)
                    fetch_qk(2)

                    for sm in range(NKC):
                        pv = psv.tile([P, F], F32, tag="psv", name="pv")
                        for r in range(NR):
                            nc.tensor.matmul(
                                pv,
                                xt_sb[:, r, sm * P : (sm + 1) * P],
                                wv_t[:, r, :],
                                start=(r == 0),
                                stop=(r == NR - 1),
                            )
                            if r == 7:
                                weaver.emit(1)
                        nc.vector.tensor_copy(
                            v_sb[:, sm, :, 0:P],
                            pv.rearrange("p (h d) -> p h d", h=HEADS_PER_CORE),
                        )
                        weaver.emit(1)

                # ======== W1-W3: Q/K heads 1-3 ‖ attention pipeline ========
                tpB = ictx.enter_context(tc.tile_pool(name="tpB", bufs=1))
                pso = att_ctx.enter_context(
                    tc.tile_pool(name="pso", bufs=2, space="PSUM")
                )
                pst = att_ctx.enter_context(
                    tc.tile_pool(name="pst", bufs=2, space="PSUM")
                )
                weaver.add(pass2_groups(0, t_a, pso, pst))
                with ExitStack() as wctx:
                    proj = wctx.enter_context(
                        tc.tile_pool(name="proj", bufs=2, space="PSUM")
                    )
                    # W1: Q1 K1 ‖ pass2-h0 (moved fully out of S1)
                    qk_head(1, wq_t[1], qt_sb, proj, weaver)
                    qk_head(1, wk_t[1], kt_sb, proj, weaver)
                    weaver.drain()  # anything left of h0
                    # W2: Q2 K2 ‖ pass1-h1
                    t_b = tpB.tile([P, T_TOTAL], BF16, name="t_b")
                    weaver.add(pass1_chunks(1, t_b, fps))
                    fetch_qk(3)
                    qk_head(2, wq_t[2], qt_sb, proj, weaver)
                    qk_head(2, wk_t[2], kt_sb, proj, weaver)
                    # W3: Q3 K3 ‖ pass2-h1 + pass1-h2 (interleaved 1:2)
                    p2h1 = pass2_groups(1, t_b, pso, pst)
                    p1h2 = pass1_chunks(2, t_a, fps)
                    inter = []
                    i1 = i2 = 0
                    while i1 < len(p2h1) or i2 < len(p1h2):
                        if i2 < len(p1h2):
                            inter.append(p1h2[i2]); i2 += 1
                        if i2 < len(p1h2):
                            inter.append(p1h2[i2]); i2 += 1
                        if i1 < len(p2h1):
                            inter.append(p2h1[i1]); i1 += 1
                    weaver.add(inter)
                    qk_head(3, wq_t[3], qt_sb, proj, weaver, per_slot=2)
                    qk_head(3, wk_t[3], kt_sb, proj, weaver, per_slot=2)
                    weaver.drain()
                pa_ctx.close()  # xt no longer needed
                # wo staged here: its SBUF slot only fits after xt is freed
                cw = ictx.enter_context(tc.tile_pool(name="cw", bufs=1))
                wo_sb = cw.tile([P, HEADS_PER_CORE, DM], BF16)
                nc.sync.dma_start(out=wo_sb[:, 0:2, :], in_=wo_r[:, 0:2, :])
                nc.sync.dma_start(out=wo_sb[:, 2:4, :], in_=wo_r[:, 2:4, :])

                # ======== W4: pass2-h2 ‖ pass1-h3 ========
                with ExitStack() as w4ctx:
                    fps2 = w4ctx.enter_context(
                        tc.tile_pool(name="fps2", bufs=1, space="PSUM")
                    )

                    class AltPool:
                        # alternate chunks between the still-open fps pool and
                        # fps2: double-buffering across pools, and chunk 0 never
                        # waits on the proj-pool bank release at the W3/W4 seam
                        def __init__(self):
                            self.i = 0

                        def tile(self, *a, **kw):
                            self.i += 1
                            return (fps if self.i % 2 else fps2).tile(*a, **kw)

                    # first half of the groups use the already-open pso ring so
                    # nothing waits on the proj-pool release at the W3/W4 seam
                    p2h2 = pass2_groups(2, t_a, pso, pst)
                    p1h3 = pass1_chunks(3, t_b, AltPool())
                    i1 = i2 = 0
                    while i1 < len(p2h2) or i2 < len(p1h3):
                        if i1 < len(p2h2):
                            p2h2[i1](); i1 += 1
                        while i2 < len(p1h3) and i2 * len(p2h2) <= i1 * len(p1h3):
                            p1h3[i2](); i2 += 1

                fps_ctx.close()

                # ======== W5: pass2-h3 ‖ output projection ========
                with ExitStack() as cctx:
                    pco = cctx.enter_context(
                        tc.tile_pool(name="pco", bufs=2, space="PSUM")
                    )
                    stg = cctx.enter_context(tc.tile_pool(name="stg", bufs=3))

                    c_stage = {}

                    def c_half(sm, half, split=1):
                        po = pco.tile([P, 1024], F32, tag="pco", name="pco_t")
                        for ff in range(HEADS_PER_CORE):
                            lhsT = ot_sb[:, ff, sm * P : (sm + 1) * P]
                            for nd in range(2):
                                n0 = half * 1024 + nd * 512
                                nc.tensor.matmul(
                                    po[:, nd * 512 : (nd + 1) * 512],
                                    lhsT,
                                    wo_sb[:, ff, n0 : n0 + 512],
                                    start=(ff == 0),
                                    stop=(ff == HEADS_PER_CORE - 1),
                                )
                        # both halves stage into one row tile: 1 out-DMA per sm
                        # (halves the DMA-setup count; the queue drains faster)
                        if sm not in c_stage:
                            c_stage[sm] = stg.tile(
                                [P, DM], BF16, tag="stage", name="stage"
                            )
                        seg = c_stage[sm][:, half * 1024 : (half + 1) * 1024]
                        if half == 0:
                            nc.vector.tensor_copy(seg, po)
                        else:
                            nc.scalar.copy(seg, po)
                        if split > 1:
                            # final row: each half DMAs itself right after its
                            # own copy, shortening the drain tail
                            nc.scalar.dma_start(
                                out=out_ap[
                                    sm * P : (sm + 1) * P,
                                    half * 1024 : (half + 1) * 1024,
                                ],
                                in_=seg,
                            )
                        elif half == 1:
                            nc.scalar.dma_start(
                                out=out_ap[sm * P : (sm + 1) * P, :],
                                in_=c_stage[sm],
                            )

                    # one eligible C-half after every pass2-h3 group keeps the
                    # PE fed between groups (pso ring); 2 halves unlock per quad
                    p2h3 = pass2_groups(3, t_b, pso, pst)
                    ready = []
                    for qb in range(NKC):
                        p2h3[qb]()
                        if qb % 4 == 3:
                            sm = qb // 4
                            ready += [(sm, 0), (sm, 1)]
                        if ready:
                            c_half(*ready.pop(0))
                    for sm, half in ready:
                        c_half(sm, half)
                    for u in range(8, 2 * NKC):
                        c_half(u // 2, u % 2, split=2 if u >= 2 * NKC - 2 else 1)
                att_ctx.close()

    nc.compile()
    return nc


def prep_in_maps(x, mask, w_q, w_k, w_v, w_o):
    """Host-side sharding: per-core input dicts (8 cores)."""
    x = np.asarray(x, dtype=np.float32)
    mask = np.asarray(mask, dtype=np.float32)
    w_q = np.asarray(w_q, dtype=np.float32)
    w_k = np.asarray(w_k, dtype=np.float32)
    w_v = np.asarray(w_v, dtype=np.float32)
    w_o = np.asarray(w_o, dtype=np.float32)

    # tri[k, q] = 1 where allowed (k <= q), from the mask's diagonal block
    tri = np.ascontiguousarray(
        (mask[:P, :P].T == 0.0).astype(NPBF16)
    )
    xts = [np.ascontiguousarray(x[b].T).astype(NPBF16) for b in range(2)]

    def pack_heads(w):  # [DM, F] -> [4*128, DM]: row h*128+p <- w[r*128+p, h*128+f]
        a = w.reshape(NR, P, HEADS_PER_CORE, P)  # [r, p, h, f]
        return np.ascontiguousarray(
            a.transpose(2, 1, 0, 3).reshape(HEADS_PER_CORE * P, DM)
        )

    in_maps = []
    for c in range(8):
        b, j = divmod(c, 4)
        sl = slice(j * F, (j + 1) * F)
        in_maps.append(
            {
                "xt": xts[b],
                "wq": pack_heads(w_q[:, sl]).astype(NPBF16),
                "wk": pack_heads(w_k[:, sl]).astype(NPBF16),
                "wv": np.ascontiguousarray(w_v[:, sl]).astype(NPBF16),
                "wo": np.ascontiguousarray(w_o[sl, :]).astype(NPBF16),
                "tri": tri,
            }
        )
    return in_maps


def gather(results):
    """Sum the 4 partial outputs per batch element."""
    out = np.zeros((2, SEQ, DM), np.float32)
    for c in range(8):
        out[c // 4] += results[c]["out"]
    return out


_cache = threading.local()


def kernel(x, mask, w_q, w_k, w_v, w_o):
    from concourse.bass_utils import run_bass_kernel_spmd

    nc = getattr(_cache, "nc", None)
    if nc is None:
        nc = build_nc(1)
        _cache.nc = nc
    in_maps = prep_in_maps(x, mask, w_q, w_k, w_v, w_o)
    res = run_bass_kernel_spmd(nc, in_maps, core_ids=list(range(8)))
    return gather(res.results)



# revision 28
# speedup vs baseline: 1.2919x; 1.0718x over previous
"""Multi-head attention (batch=2, seq=2048, d_model=2048, 16 heads, causal)
on 8 Trainium2 NeuronCores.

Sharding (Megatron-style tensor parallel + data parallel):
  core c -> batch b = c // 4, feature block j = c % 4 (4 heads = 512 features).
  Each core computes Q/K/V projections for its 512 feature columns
  (w_q/w_k/w_v column-sliced), attention for its 4 heads, and a partial
  output projection (w_o row-sliced).  The 4 partial outputs per batch
  element are summed on the host (the Megatron row-parallel AllReduce).

v4 per-head software pipeline: ACT-engine exp (~80us serial) hides under
PE projection matmuls; a single compact triangular T buffer (ring-1)
keeps SBUF under budget.
  S0: Q-h0 + K-h0, r-outer over shared xt chunks (DMA-paced startup)
  S1: V (all heads)  ‖ pass1-h0 woven, pass2-h0 in the tail
  W1: Q-h1 K-h1      ‖ pass2-h0 finish
  W2: Q-h2 K-h2      ‖ pass1-h1
  W3: Q-h3 K-h3      ‖ pass2-h1 + pass1-h2
  W4: pass2-h2       ‖ pass1-h3
  W5: pass2-h3       ‖ output projection (interleaved, split DMA tail)
All matmuls bf16 with fp32 PSUM accumulation; unnormalized softmax with
a fused ones-column denominator in V (scores are O(5), fp32 exp: no max
shift needed).  wq/wk are host-packed per-head ([4*128, 2048]) so each
head's weight slice is one contiguous DMA.
"""

import math
import threading
from contextlib import ExitStack

import ml_dtypes
import numpy as np

import concourse.bass as bass
import concourse.mybir as mybir
import concourse.tile as tile
from concourse import bacc
from concourse.masks import make_identity

BF16 = mybir.dt.bfloat16
F32 = mybir.dt.float32
NPBF16 = ml_dtypes.bfloat16

SEQ = 2048
DM = 2048
HEADS_PER_CORE = 4
F = 512  # features per core
P = 128
NKC = SEQ // P  # 16 key blocks
NR = DM // P  # 16 contraction chunks
SCALE = 1.0 / math.sqrt(128.0)

# compact T-buffer offsets: block kc covers q in [kc*128, 2048)
T_WIDTHS = [SEQ - kc * P for kc in range(NKC)]
T_OFFS = list(np.cumsum([0] + T_WIDTHS[:-1]))
T_TOTAL = int(np.sum(T_WIDTHS))  # 17408


class Weaver:
    """Paced round-robin emitter of chunk closures."""

    def __init__(self):
        self.chunks = []
        self.pos = 0

    def add(self, chunks):
        self.chunks.extend(chunks)

    def emit(self, n=1):
        e = 0
        while self.pos < len(self.chunks) and e < n:
            self.chunks[self.pos]()
            self.pos += 1
            e += 1

    def pending(self):
        return len(self.chunks) - self.pos

    def drain(self):
        self.emit(self.pending())


def build_nc(iters: int = 1, **_ignored) -> bacc.Bacc:
    nc = bacc.Bacc("TRN2", num_devices=8)

    xt_h = nc.dram_tensor("xt", [DM, SEQ], BF16, kind="ExternalInput")
    # wq/wk packed per head: row h*128+p, col r*128+f  <-  w[r*128+p, h*128+f]
    wq_h = nc.dram_tensor("wq", [HEADS_PER_CORE * P, DM], BF16, kind="ExternalInput")
    wk_h = nc.dram_tensor("wk", [HEADS_PER_CORE * P, DM], BF16, kind="ExternalInput")
    wv_h = nc.dram_tensor("wv", [DM, F], BF16, kind="ExternalInput")
    wo_h = nc.dram_tensor("wo", [F, DM], BF16, kind="ExternalInput")
    tri_h = nc.dram_tensor("tri", [P, P], BF16, kind="ExternalInput")
    out_h = nc.dram_tensor("out", [SEQ, DM], BF16, kind="ExternalOutput")

    xt = xt_h.ap()
    wq_ap = wq_h.ap()
    wk_ap = wk_h.ap()
    wo_r = wo_h.ap().rearrange("(c p) n -> p c n", p=P)  # [128, 4, 2048]
    out_ap = out_h.ap()

    with tile.TileContext(nc) as tc, ExitStack() as octx:
        consts = octx.enter_context(tc.tile_pool(name="consts", bufs=1))
        ident = consts.tile([P, P], BF16)
        make_identity(nc, ident)
        tri_sb = consts.tile([P, P], BF16)
        nc.sync.dma_start(out=tri_sb, in_=tri_h.ap())

        for _ in range(iters):
            with ExitStack() as ictx:
                persist = ictx.enter_context(tc.tile_pool(name="persist", bufs=1))
                qt_sb = persist.tile([P, HEADS_PER_CORE, SEQ], BF16)
                kt_sb = persist.tile([P, HEADS_PER_CORE, SEQ], BF16)
                v_sb = persist.tile([P, NKC, HEADS_PER_CORE, P + 1], BF16)
                ot_sb = persist.tile([P, HEADS_PER_CORE, SEQ], BF16)
                nc.vector.memset(v_sb[:, :, :, P : P + 1], 1.0)
                pbo = ictx.enter_context(tc.tile_pool(name="pbo", bufs=3))
                tp = ictx.enter_context(tc.tile_pool(name="tp", bufs=1))
                wqk = ictx.enter_context(tc.tile_pool(name="wqk", bufs=1))
                pa_ctx = ExitStack()
                pa = pa_ctx.enter_context(
                    tc.tile_pool(name="pa", bufs=1, side="right")
                )
                xt_sb = pa.tile([P, NR, SEQ], BF16)

                # ---------------- emitters ----------------
                def pass1_chunks(h, t_t, fps_pool, width=1024):
                    """T = causal_mask(exp(S^T/sqrt(d))), compact kc-major
                    chunks with kt-weight reuse."""
                    chunks = []
                    for kc in range(NKC):
                        w = T_WIDTHS[kc]
                        for c in range((w + width - 1) // width):
                            def emit(kc=kc, c=c, w=w):
                                off = T_OFFS[kc]
                                q0 = kc * P
                                lhsT = kt_sb[:, h, kc * P : (kc + 1) * P]
                                wc = min(width, w - c * width)
                                ps = fps_pool.tile(
                                    [P, width], F32, tag="fps", name="fps"
                                )
                                for n in range((wc + 511) // 512):
                                    nw = min(512, wc - n * 512)
                                    o0 = c * width + n * 512
                                    nc.tensor.matmul(
                                        ps[:, n * 512 : n * 512 + nw],
                                        lhsT,
                                        qt_sb[:, h, q0 + o0 : q0 + o0 + nw],
                                        start=True,
                                        stop=True,
                                    )
                                nc.scalar.activation(
                                    t_t[:, off + c * width : off + c * width + wc],
                                    ps[:, 0:wc],
                                    mybir.ActivationFunctionType.Exp,
                                    scale=SCALE,
                                )
                                if c == 0:
                                    nc.vector.tensor_mul(
                                        t_t[:, off : off + P],
                                        t_t[:, off : off + P],
                                        tri_sb,
                                    )
                            chunks.append(emit)
                    return chunks

                def pass2_groups(h, t_t, pso_pool, pst_pool):
                    """AV + normalize + transpose per query block.
                    pso_pool may be a pool or a qb->pool function."""
                    state = {}
                    groups = []
                    for qb in range(NKC):
                        def emit(qb=qb):
                            pp = pso_pool(qb) if callable(pso_pool) else pso_pool
                            po = pp.tile([P, P + 1], F32, tag="pso", name="po")
                            for kc in range(qb + 1):
                                col = T_OFFS[kc] + (qb - kc) * P
                                nc.tensor.matmul(
                                    po,
                                    t_t[:, col : col + P],
                                    v_sb[:, kc, h, :],
                                    start=(kc == 0),
                                    stop=(kc == qb),
                                )
                            recip = pbo.tile([P, 1], F32, tag="recip", name="recip")
                            nc.vector.reciprocal(recip, po[:, P : P + 1])
                            o_sb = pbo.tile([P, P], BF16, tag="o", name="o_sb")
                            nc.vector.tensor_scalar_mul(o_sb, po[:, 0:P], recip)
                            i4 = qb % 4
                            if i4 == 0:
                                state["pt"] = pst_pool.tile(
                                    [P, 512], BF16, tag="pst", name="pt"
                                )
                            nc.tensor.transpose(
                                state["pt"][:, i4 * P : (i4 + 1) * P], o_sb, ident
                            )
                            if i4 == 3:
                                g = qb // 4
                                nc.vector.tensor_copy(
                                    ot_sb[:, h, g * 512 : (g + 1) * 512],
                                    state["pt"],
                                )
                        groups.append(emit)
                    return groups

                def qk_head(h, w_t, dst, proj_pool, weaver, per_slot=1):
                    """Steady-state projection for one head: 512-col quarters
                    (1-bank psum tiles), r-inner, weaving attention chunks
                    between r groups."""
                    for quarter in range(4):
                        pk = proj_pool.tile([P, 512], F32, tag="proj", name="pk")
                        s0 = quarter * 512
                        for r in range(NR):
                            nc.tensor.matmul(
                                pk,
                                w_t[:, r * P : (r + 1) * P],
                                xt_sb[:, r, s0 : s0 + 512],
                                start=(r == 0),
                                stop=(r == NR - 1),
                            )
                            if r % 5 == 4:
                                weaver.emit(per_slot)
                        nc.vector.tensor_copy(dst[:, h, s0 : s0 + 512], pk)
                        weaver.emit(per_slot)

                # ---------------- DMA schedule ----------------
                wq_t = [None] * HEADS_PER_CORE
                wk_t = [None] * HEADS_PER_CORE

                def fetch_qk(h):
                    wq_t[h] = wqk.tile([P, DM], BF16, tag="wq", name="wq_t")
                    nc.sync.dma_start(
                        out=wq_t[h], in_=wq_ap[h * P : (h + 1) * P, :]
                    )
                    wk_t[h] = wqk.tile([P, DM], BF16, tag="wk", name="wk_t")
                    nc.sync.dma_start(
                        out=wk_t[h], in_=wk_ap[h * P : (h + 1) * P, :]
                    )

                # wq -> xt0 -> wk, then per-chunk xt DMAs
                wq_t[0] = wqk.tile([P, DM], BF16, tag="wq", name="wq_t")
                nc.sync.dma_start(out=wq_t[0], in_=wq_ap[0:P, :])
                nc.sync.dma_start(out=xt_sb[:, 0, :], in_=xt[0:P, :])
                wk_t[0] = wqk.tile([P, DM], BF16, tag="wk", name="wk_t")
                nc.sync.dma_start(out=wk_t[0], in_=wk_ap[0:P, :])
                for r in range(1, NR):
                    nc.sync.dma_start(
                        out=xt_sb[:, r, :], in_=xt[r * P : (r + 1) * P, :]
                    )

                # ======== S0: Q-h0 full + K-h0 half0, r-outer (DMA-paced) ====
                # Q psum on the left PSUM stack, K-half on the right: V's psum
                # (right, below) then never overlaps a zone gated on S0 copies,
                # so V starts with no seam.  K-half1 is woven into V.
                with ExitStack() as s0ctx:
                    ps0q = s0ctx.enter_context(
                        tc.tile_pool(name="ps0q", bufs=1, space="PSUM")
                    )
                    ps0k = s0ctx.enter_context(
                        tc.tile_pool(name="ps0k", bufs=1, space="PSUM", side="right")
                    )
                    pq = ps0q.tile([P, SEQ], F32, tag="ps0q", name="pq")
                    pk0 = ps0k.tile([P, 1024], F32, tag="ps0k", name="pk0")
                    for r in range(NR):
                        lhsT = wq_t[0][:, r * P : (r + 1) * P]
                        for sn in range(4):
                            nc.tensor.matmul(
                                pq[:, sn * 512 : (sn + 1) * 512],
                                lhsT,
                                xt_sb[:, r, sn * 512 : (sn + 1) * 512],
                                start=(r == 0),
                                stop=(r == NR - 1),
                            )
                        lhsT = wk_t[0][:, r * P : (r + 1) * P]
                        for sn in range(2):
                            nc.tensor.matmul(
                                pk0[:, sn * 512 : (sn + 1) * 512],
                                lhsT,
                                xt_sb[:, r, sn * 512 : (sn + 1) * 512],
                                start=(r == 0),
                                stop=(r == NR - 1),
                            )
                    # kt first (pass1-h0 needs it)
                    nc.vector.tensor_copy(kt_sb[:, 0, 0:1024], pk0)
                    nc.vector.tensor_copy(qt_sb[:, 0, 0:1024], pq[:, 0:1024])
                    nc.scalar.copy(qt_sb[:, 0, 1024:2048], pq[:, 1024:2048])

                # attention-era PSUM pools; pso/pst open after S1 (both
                # ring-2: po and pst chains each double-buffered)
                att_ctx = ExitStack()
                fps_ctx = ExitStack()
                fps = fps_ctx.enter_context(
                    tc.tile_pool(name="fps", bufs=1, space="PSUM", side="right")
                )

                # two T buffers alternate across heads (h0,h2 -> tA; h1,h3 ->
                # tB): pass1-h(i+1) then never overwrites what pass2-h(i) is
                # still reading
                t_a = tp.tile([P, T_TOTAL], BF16, tag="T", name="t_a", bufs=1)
                weaver = Weaver()

                # ======== S1: V (all heads) ‖ K-h0 half1 ‖ pass1-h0 ========
                with ExitStack() as vctx:
                    psk1 = vctx.enter_context(
                        tc.tile_pool(name="psk1", bufs=1, space="PSUM", side="right")
                    )
                    psv = vctx.enter_context(
                        tc.tile_pool(name="psv", bufs=2, space="PSUM", side="right")
                    )
                    wvp = vctx.enter_context(tc.tile_pool(name="wvp", bufs=1))

                    kh1_state = {}

                    def kh1_chunk(g):
                        def emit():
                            if g == 0:
                                kh1_state["pk"] = psk1.tile(
                                    [P, 1024], F32, tag="psk1", name="pk1"
                                )
                            pk1 = kh1_state["pk"]
                            for r in range(4 * g, 4 * g + 4):
                                lhsT = wk_t[0][:, r * P : (r + 1) * P]
                                for sn in range(2):
                                    nc.tensor.matmul(
                                        pk1[:, sn * 512 : (sn + 1) * 512],
                                        lhsT,
                                        xt_sb[
                                            :, r, 1024 + sn * 512 : 1024 + (sn + 1) * 512
                                        ],
                                        start=(r == 0),
                                        stop=(r == NR - 1),
                                    )
                            if g == 3:
                                nc.vector.tensor_copy(
                                    kt_sb[:, 0, 1024:2048], pk1
                                )
                        return emit

                    p1h0 = pass1_chunks(0, t_a, fps)
                    inter0 = []
                    for g in range(4):
                        inter0.append(kh1_chunk(g))
                        inter0 += p1h0[4 * g : 4 * g + 4]
                    inter0 += p1h0[16:]
                    weaver.add(inter0)
                    wv_r = wv_h.ap().rearrange("(r p) f -> p r f", p=P)
                    wv_t = wvp.tile([P, NR, F], BF16)
                    for rr in range(4):
                        nc.sync.dma_start(
                            out=wv_t[:, rr * 4 : (rr + 1) * 4, :],
                            in_=wv_r[:, rr * 4 : (rr + 1) * 4, :],
                        )
                    # prefetch weights for heads 1-2 (ring-gated on h0/h1 release;
                    # queued after wv/wo so they can't head-of-line-block them)
                    fetch_qk(1)
                    fetch_qk(2)

                    for sm in range(NKC):
                        pv = psv.tile([P, F], F32, tag="psv", name="pv")
                        for r in range(NR):
                            nc.tensor.matmul(
                                pv,
                                xt_sb[:, r, sm * P : (sm + 1) * P],
                                wv_t[:, r, :],
                                start=(r == 0),
                                stop=(r == NR - 1),
                            )
                            if r == 7:
                                weaver.emit(1)
                        nc.vector.tensor_copy(
                            v_sb[:, sm, :, 0:P],
                            pv.rearrange("p (h d) -> p h d", h=HEADS_PER_CORE),
                        )
                        weaver.emit(1)

                # ======== W1-W3: Q/K heads 1-3 ‖ attention pipeline ========
                tpB = ictx.enter_context(tc.tile_pool(name="tpB", bufs=1))
                pso = att_ctx.enter_context(
                    tc.tile_pool(name="pso", bufs=2, space="PSUM")
                )
                pst = att_ctx.enter_context(
                    tc.tile_pool(name="pst", bufs=2, space="PSUM")
                )
                weaver.add(pass2_groups(0, t_a, pso, pst))
                with ExitStack() as wctx:
                    proj = wctx.enter_context(
                        tc.tile_pool(name="proj", bufs=2, space="PSUM")
                    )
                    # W1: Q1 K1 ‖ pass2-h0 (moved fully out of S1)
                    qk_head(1, wq_t[1], qt_sb, proj, weaver)
                    qk_head(1, wk_t[1], kt_sb, proj, weaver)
                    weaver.drain()  # anything left of h0
                    # W2: Q2 K2 ‖ pass1-h1
                    t_b = tpB.tile([P, T_TOTAL], BF16, name="t_b")
                    weaver.add(pass1_chunks(1, t_b, fps))
                    fetch_qk(3)
                    qk_head(2, wq_t[2], qt_sb, proj, weaver)
                    qk_head(2, wk_t[2], kt_sb, proj, weaver)
                    # W3: Q3 K3 ‖ pass2-h1 + pass1-h2 (interleaved 1:2)
                    p2h1 = pass2_groups(1, t_b, pso, pst)
                    p1h2 = pass1_chunks(2, t_a, fps)
                    inter = []
                    i1 = i2 = 0
                    while i1 < len(p2h1) or i2 < len(p1h2):
                        if i2 < len(p1h2):
                            inter.append(p1h2[i2]); i2 += 1
                        if i2 < len(p1h2):
                            inter.append(p1h2[i2]); i2 += 1
                        if i1 < len(p2h1):
                            inter.append(p2h1[i1]); i1 += 1
                    weaver.add(inter)
                    qk_head(3, wq_t[3], qt_sb, proj, weaver, per_slot=2)
                    qk_head(3, wk_t[3], kt_sb, proj, weaver, per_slot=2)
                    weaver.drain()
                pa_ctx.close()  # xt no longer needed
                # wo staged here: its SBUF slot only fits after xt is freed
                cw = ictx.enter_context(tc.tile_pool(name="cw", bufs=1))
                wo_sb = cw.tile([P, HEADS_PER_CORE, DM], BF16)
                nc.sync.dma_start(out=wo_sb[:, 0:2, :], in_=wo_r[:, 0:2, :])
                nc.sync.dma_start(out=wo_sb[:, 2:4, :], in_=wo_r[:, 2:4, :])

                # ======== W4: pass2-h2 ‖ pass1-h3 ========
                with ExitStack() as w4ctx:
                    fps2 = w4ctx.enter_context(
                        tc.tile_pool(name="fps2", bufs=1, space="PSUM")
                    )

                    class AltPool:
                        # alternate chunks between the still-open fps pool and
                        # fps2: double-buffering across pools, and chunk 0 never
                        # waits on the proj-pool bank release at the W3/W4 seam
                        def __init__(self):
                            self.i = 0

                        def tile(self, *a, **kw):
                            self.i += 1
                            return (fps if self.i % 2 else fps2).tile(*a, **kw)

                    # first half of the groups use the already-open pso ring so
                    # nothing waits on the proj-pool release at the W3/W4 seam
                    p2h2 = pass2_groups(2, t_a, pso, pst)
                    p1h3 = pass1_chunks(3, t_b, AltPool())
                    i1 = i2 = 0
                    while i1 < len(p2h2) or i2 < len(p1h3):
                        if i1 < len(p2h2):
                            p2h2[i1](); i1 += 1
                        while i2 < len(p1h3) and i2 * len(p2h2) <= i1 * len(p1h3):
                            p1h3[i2](); i2 += 1

                fps_ctx.close()

                # ======== W5: pass2-h3 ‖ output projection ========
                with ExitStack() as cctx:
                    pco = cctx.enter_context(
                        tc.tile_pool(name="pco", bufs=2, space="PSUM")
                    )
                    stg = cctx.enter_context(tc.tile_pool(name="stg", bufs=3))

                    c_stage = {}

                    def c_half(sm, half, split=1):
                        po = pco.tile([P, 1024], F32, tag="pco", name="pco_t")
                        for ff in range(HEADS_PER_CORE):
                            lhsT = ot_sb[:, ff, sm * P : (sm + 1) * P]
                            for nd in range(2):
                                n0 = half * 1024 + nd * 512
                                nc.tensor.matmul(
                                    po[:, nd * 512 : (nd + 1) * 512],
                                    lhsT,
                                    wo_sb[:, ff, n0 : n0 + 512],
                                    start=(ff == 0),
                                    stop=(ff == HEADS_PER_CORE - 1),
                                )
                        # both halves stage into one row tile: 1 out-DMA per sm
                        # (halves the DMA-setup count; the queue drains faster)
                        if sm not in c_stage:
                            c_stage[sm] = stg.tile(
                                [P, DM], BF16, tag="stage", name="stage"
                            )
                        seg = c_stage[sm][:, half * 1024 : (half + 1) * 1024]
                        if half == 0:
                            nc.vector.tensor_copy(seg, po)
                        else:
                            nc.scalar.copy(seg, po)
                        if split > 1:
                            # final row: each half DMAs itself right after its
                            # own copy, shortening the drain tail
                            nc.scalar.dma_start(
                                out=out_ap[
                                    sm * P : (sm + 1) * P,
                                    half * 1024 : (half + 1) * 1024,
                                ],
                                in_=seg,
                            )
                        elif half == 1:
                            nc.scalar.dma_start(
                                out=out_ap[sm * P : (sm + 1) * P, :],
                                in_=c_stage[sm],
                            )

                    # one eligible C-half after every pass2-h3 group keeps the
                    # PE fed between groups (pso ring); 2 halves unlock per quad
                    p2h3 = pass2_groups(3, t_b, pso, pst)
                    ready = []
                    for qb in range(NKC):
                        p2h3[qb]()
                        if qb % 4 == 3:
                            sm = qb // 4
                            ready += [(sm, 0), (sm, 1)]
                        if ready:
                            c_half(*ready.pop(0))
                    for sm, half in ready:
                        c_half(sm, half)
                    for u in range(8, 2 * NKC):
                        c_half(u // 2, u % 2, split=2 if u >= 2 * NKC - 2 else 1)
                att_ctx.close()

    nc.compile()
    return nc


def prep_in_maps(x, mask, w_q, w_k, w_v, w_o):
    """Host-side sharding: per-core input dicts (8 cores)."""
    x = np.asarray(x, dtype=np.float32)
    mask = np.asarray(mask, dtype=np.float32)
    w_q = np.asarray(w_q, dtype=np.float32)
    w_k = np.asarray(w_k, dtype=np.float32)
    w_v = np.asarray(w_v, dtype=np.float32)
    w_o = np.asarray(w_o, dtype=np.float32)

    # tri[k, q] = 1 where allowed (k <= q), from the mask's diagonal block
    tri = np.ascontiguousarray(
        (mask[:P, :P].T == 0.0).astype(NPBF16)
    )
    xts = [np.ascontiguousarray(x[b].T).astype(NPBF16) for b in range(2)]

    def pack_heads(w):  # [DM, F] -> [4*128, DM]: row h*128+p <- w[r*128+p, h*128+f]
        a = w.reshape(NR, P, HEADS_PER_CORE, P)  # [r, p, h, f]
        return np.ascontiguousarray(
            a.transpose(2, 1, 0, 3).reshape(HEADS_PER_CORE * P, DM)
        )

    in_maps = []
    for c in range(8):
        b, j = divmod(c, 4)
        sl = slice(j * F, (j + 1) * F)
        in_maps.append(
            {
                "xt": xts[b],
                "wq": pack_heads(w_q[:, sl]).astype(NPBF16),
                "wk": pack_heads(w_k[:, sl]).astype(NPBF16),
                "wv": np.ascontiguousarray(w_v[:, sl]).astype(NPBF16),
                "wo": np.ascontiguousarray(w_o[sl, :]).astype(NPBF16),
                "tri": tri,
            }
        )
    return in_maps


def gather(results):
    """Sum the 4 partial outputs per batch element."""
    out = np.zeros((2, SEQ, DM), np.float32)
    for c in range(8):
        out[c // 4] += results[c]["out"]
    return out


_cache = threading.local()


def kernel(x, mask, w_q, w_k, w_v, w_o):
    from concourse.bass_utils import run_bass_kernel_spmd

    nc = getattr(_cache, "nc", None)
    if nc is None:
        nc = build_nc(1)
        _cache.nc = nc
    in_maps = prep_in_maps(x, mask, w_q, w_k, w_v, w_o)
    res = run_bass_kernel_spmd(nc, in_maps, core_ids=list(range(8)))
    return gather(res.results)

